# revision 38
# baseline (speedup 1.0000x reference)
"""Trainium2 Bass kernel v3: causal spatial attention block,
minimal-instruction design (f32 everywhere except the fp8-DoubleRow
q/k/scores path).

Data-parallel over batch across 8 NeuronCores (4 batches/core, no
collectives). The execution backend charges a large, mostly
size-independent cost per *instruction* (ACT worst ~250us; MM ~50us;
DVE/DMA ~30-100us), so this kernel minimizes instruction count:
  - Q/K projections (fp8-DR): 32 matmuls total for all 4 batches
    (contraction 256 per instr); fused bias+relu drains write fp8 q/k
    directly -> 4 DVE instructions.
  - V projection per batch (f32, accuracy-critical): 16 matmuls produce
    vT directly in [t, v] layout (lhsT = x-tile, rhs = wv); the free-dim
    bias + relu run as two DVE passes against a broadcast const (cheaper
    than rank-1 bias matmuls on the PE).
  - scores per batch (fp8-DR): per (t-tile, 512-chunk): 1 DR k-matmul +
    1 additive-mask matmul (identity-lhsT x shifted window into one
    [128,1920] E-pattern const); fully-masked chunks are skipped
    entirely (their p region is never contracted by the causally
    trimmed o/dn). exp(score/16) runs as ONE activation per 4-t-tile
    phase over the whole [128,4,1024] PSUM (2 per batch); masked
    entries underflow to exact f32 zeros.
  - o/dn per batch (f32): 36 matmuls, causally trimmed; reciprocal + 2
    multiplies normalize during the PSUM drain into an all-batch output
    tile shipped by ONE DMA per rep (transposed [m,p,b,s] DRAM layout,
    un-transposed on the host).
All PSUM phases share one [128,4096] tile (8 banks), start=True only on
the first matmul touching each 512-col bank. ~470 instructions per
4-batch pass vs 1227 for the previous fp8-everything kernel (77.5ms ->
~21ms measured by reps-delta wall timing; NTFF profiling wedges the
axon terminal).
"""
import numpy as np
from contextlib import ExitStack

import concourse.bass as bass
import concourse.mybir as mybir
import concourse.tile as tile
from concourse import bacc
from concourse.bass_utils import run_bass_kernel_spmd

F32 = mybir.dt.float32
FP8 = mybir.dt.float8e4
AF = mybir.ActivationFunctionType
ALU = mybir.AluOpType
DR = mybir.MatmulPerfMode.DoubleRow

B, C, L, EMB = 32, 224, 32, 16
S = L * L            # 1024
CIN = 256
NCORES = 8
NB = B // NCORES     # 4 batches per core
SB = NB * S          # 4096 columns: all 4 batches side by side
MASKV = -30000.0     # exp((x-30000)/16) underflows to exact f32 zero


def _pos_embeddings() -> np.ndarray:
    pos = np.arange(L)[:, None].astype(np.float64)
    j = np.arange(EMB)[None, :]
    enc = pos / np.power(10000.0, 2.0 * (j // 2) / EMB)
    enc[0, :] = 0.0
    enc[1:, 0::2] = np.sin(enc[1:, 0::2])
    enc[1:, 1::2] = np.cos(enc[1:, 1::2])
    t = enc.astype(np.float32)
    x = np.tile(t.reshape(1, EMB, L, 1), (1, 1, 1, L))
    y = np.tile(t.reshape(1, EMB, 1, L), (1, 1, L, 1))
    pe = np.concatenate((x, y), axis=1)[0]
    return np.ascontiguousarray(pe.reshape(2 * EMB, S))


def build(reps: int = 1):
    nc = bacc.Bacc("TRN2", target_bir_lowering=False, debug=False,
                   num_devices=NCORES)
    x_d = nc.declare_dram_parameter("xm", [2, 128, NB, S], F32, isOutput=False)
    wq_d = nc.declare_dram_parameter("wqt", [CIN, 256], F32, isOutput=False)
    wk_d = nc.declare_dram_parameter("wkt", [CIN, 256], F32, isOutput=False)
    wv_d = nc.declare_dram_parameter("wvt", [CIN, 256], F32, isOutput=False)
    bqk_d = nc.declare_dram_parameter("bqk", [128, 4], F32, isOutput=False)
    bv_d = nc.declare_dram_parameter("bvb", [128, 2048], F32, isOutput=False)
    # mconst: [ identity(128) | E-mask(1920) ]  (E[p][j] = MASKV if j < 896+p)
    mk_d = nc.declare_dram_parameter("mconst", [128, 2048], F32, isOutput=False)
    out_d = nc.declare_dram_parameter("out", [2, 128, NB, S], F32,
                                      isOutput=True)

    with ExitStack() as ctx:
        tc = ctx.enter_context(tile.TileContext(nc))
        const = ctx.enter_context(tc.tile_pool(name="const", bufs=1))
        xp = ctx.enter_context(tc.tile_pool(name="xp", bufs=1))
        qkp = ctx.enter_context(tc.tile_pool(name="qk", bufs=1))
        vtp = ctx.enter_context(tc.tile_pool(name="vt", bufs=2))
        ppool = ctx.enter_context(tc.tile_pool(name="pp", bufs=1))
        op = ctx.enter_context(tc.tile_pool(name="ob", bufs=2))
        rp = ctx.enter_context(tc.tile_pool(name="rp", bufs=2))
        psp = ctx.enter_context(tc.tile_pool(name="ps", bufs=1, space="PSUM"))

        # ---- constants ----
        # SWDGE casting DMAs: f32 DRAM -> fp8 SBUF, bit-exact RNE
        wqt = const.tile([128, 2, 256], FP8, tag="wqt")
        nc.gpsimd.dma_start(wqt[:],
                            wq_d[:].rearrange("(ci p) m -> p ci m", p=128))
        wkt = const.tile([128, 2, 256], FP8, tag="wkt")
        nc.gpsimd.dma_start(wkt[:],
                            wk_d[:].rearrange("(ci p) m -> p ci m", p=128))
        wvt = const.tile([128, 2, 256], F32, tag="wvt")
        nc.sync.dma_start(wvt[:], wv_d[:].rearrange("(ci p) m -> p ci m", p=128))
        bqk = const.tile([128, 4], F32, tag="bqk")
        nc.sync.dma_start(bqk[:], bqk_d[:])
        bvb = const.tile([128, 2048], F32, tag="bvb")
        nc.sync.dma_start(bvb[:], bv_d[:])
        mck = const.tile([128, 2048], F32, tag="mck")
        nc.sync.dma_start(mck[:], mk_d[:])
        ones128 = const.tile([128, 128], F32, tag="ones128")
        nc.vector.memset(ones128[:], 1.0)
        ident = mck[:, 0:128]
        emask = mck[:, 128:2048]           # [128, 1920]

        ps = psp.tile([128, 4096], F32, tag="ps")

        for rep in range(reps):
            # ---- x load: all 4 batches -> [128, 2(ci), 4096] ----
            xt = xp.tile([128, 2, SB], F32, tag="xt")
            nc.sync.dma_start(xt[:], x_d[:].rearrange("ci p b s -> p ci (b s)"))
            # fp8 copy of x for the Q/K path (SWDGE casting DMA)
            xt8 = xp.tile([128, 2, SB], FP8, tag="xt8")
            nc.gpsimd.dma_start(xt8[:],
                                x_d[:].rearrange("ci p b s -> p ci (b s)"))

            # ---- Q/K projections (fp8 DoubleRow), all batches at once ----
            q = qkp.tile([128, 2, SB], FP8, tag="q")
            k = qkp.tile([128, 2, SB], FP8, tag="k")
            for pi, (wt, dst) in enumerate(((wqt, q), (wkt, k))):
                for m in range(2):
                    for c in range(8):
                        cs = slice(512 * c, 512 * c + 512)
                        nc.tensor.matmul(ps[:, cs],
                                         wt[:, :, 128 * m:128 * m + 128],
                                         xt8[:, :, cs], start=True, stop=True,
                                         perf_mode=DR)
                    nc.vector.tensor_scalar(dst[:, m, :], ps[:],
                                            bqk[:, 2 * pi + m:2 * pi + m + 1],
                                            0.0, op0=ALU.add, op1=ALU.max)

            for b in range(NB):
                s0 = b * S

                # ---- V projection -> vT [128(t), 8, 256(v)] ----
                vt = vtp.tile([128, 8, 256], F32, tag="vt")
                for a in range(8):
                    for ci in range(2):
                        nc.tensor.matmul(
                            ps[:, 256 * a:256 * a + 256],
                            xt[:, ci, s0 + 128 * a:s0 + 128 * a + 128],
                            wvt[:, ci, :],
                            start=(a % 2 == 0 and ci == 0),
                            stop=(a % 2 == 1 and ci == 1))
                # bias (along free dim) + relu on DVE: 2 passes replace
                # 4 rank-1 bias matmuls
                nc.vector.tensor_tensor(
                    vt[:], ps[:, 0:2048].rearrange("p (a v) -> p a v", a=8),
                    bvb[:].rearrange("p (a v) -> p a v", a=8), op=ALU.add)
                nc.vector.tensor_scalar(vt[:], vt[:], 0.0, None, op0=ALU.max)

                # ---- scores + exp: 2 phases of 4 t-tiles ----
                # Fully-masked chunks (i>=4, s-chunk 0) are skipped outright:
                # the exp there reads this batch's phase-0 leftovers (finite,
                # saturates in fp8) and o/dn never contracts that region.
                pt = ppool.tile([128, 8, S], F32, tag="pt")
                for ph in range(2):
                    for j in range(4):
                        i = 4 * ph + j
                        tc_ = slice(s0 + 128 * i, 128 * i + s0 + 128)
                        for c in range(2):
                            sc = ps[:, 1024 * j + 512 * c:
                                    1024 * j + 512 * c + 512]
                            j0 = 896 - 128 * i + 512 * c
                            if j0 <= 384:
                                continue
                            has_mask = j0 < 1024
                            qs = slice(s0 + 512 * c, s0 + 512 * c + 512)
                            nc.tensor.matmul(sc, k[:, :, tc_],
                                             q[:, :, qs], start=True,
                                             stop=not has_mask,
                                             perf_mode=DR)
                            if has_mask:
                                nc.tensor.matmul(sc, ident,
                                                 emask[:, j0:j0 + 512],
                                                 start=False, stop=True)
                    nc.scalar.activation(
                        pt[:, 4 * ph:4 * ph + 4, :],
                        ps[:].rearrange("p (a s) -> p a s", a=4),
                        AF.Exp, scale=0.0625)

                # ---- o / dn ----
                om0 = ps[:, 0:1024]
                om1 = ps[:, 1024:2048]
                dnp = ps[:, 2048:3072]
                for c in range(2):
                    ntile = 4 * c + 4   # causal: chunk c only sees t < 512(c+1)
                    for i in range(ntile):
                        st, sp_ = (i == 0), (i == ntile - 1)
                        pcs = pt[:, i, 512 * c:512 * c + 512]
                        for roff, lh in ((0, vt[:, i, 0:128]),
                                         (1024, vt[:, i, 128:256]),
                                         (2048, ones128[:])):
                            nc.tensor.matmul(
                                ps[:, roff + 512 * c:roff + 512 * c + 512],
                                lh, pcs, start=st, stop=sp_)

                # ---- normalize ----
                rec = rp.tile([128, S], F32, tag="rec")
                nc.vector.reciprocal(rec[:], dnp)
                if b == 0:
                    osb = op.tile([128, 2, SB], F32, tag="osb")
                nc.vector.tensor_tensor(osb[:, 0, s0:s0 + S], om0, rec[:],
                                        op=ALU.mult)
                nc.vector.tensor_tensor(osb[:, 1, s0:s0 + S], om1, rec[:],
                                        op=ALU.mult)
            # one output DMA for all 4 batches
            nc.sync.dma_start(out_d[:].rearrange("m p b s -> p m (b s)"),
                              osb[:])

    nc.finalize()
    return nc


def make_in_maps(x, wq, bq, wk, bk, wv, bv):
    x_r = x.reshape(B, C, S).astype(np.float32)
    pe = _pos_embeddings()
    xm = np.concatenate(
        [x_r, np.broadcast_to(pe[None], (B, 2 * EMB, S))], axis=1)
    # [B, 2, 128, S] -> per-core [2, 128, NB, S] so the on-device DMA can
    # group (b s) into adjacent columns
    xm = xm.reshape(B, 2, 128, S)
    wqt = np.ascontiguousarray(wq.T.astype(np.float32))
    wkt = np.ascontiguousarray(wk.T.astype(np.float32))
    wvt = np.ascontiguousarray(wv.T.astype(np.float32))
    bq = bq.astype(np.float32)
    bk = bk.astype(np.float32)
    bqk = np.ascontiguousarray(
        np.stack([bq[:128], bq[128:], bk[:128], bk[128:]], axis=1))
    bvb = np.ascontiguousarray(np.broadcast_to(
        np.tile(bv.astype(np.float32), 8)[None, :], (128, 2048)))
    ident = np.eye(128, dtype=np.float32)
    jj = np.arange(1920)[None, :]
    ppn = np.arange(128)[:, None]
    emask = np.where(jj < 896 + ppn, np.float32(MASKV), np.float32(0.0))
    mconst = np.ascontiguousarray(
        np.concatenate([ident, emask.astype(np.float32)], axis=1))
    common = dict(wqt=wqt, wkt=wkt, wvt=wvt, bqk=bqk, bvb=bvb, mconst=mconst)
    return [dict(xm=np.ascontiguousarray(
                     xm[i * NB:(i + 1) * NB].transpose(1, 2, 0, 3)), **common)
            for i in range(NCORES)]


_NC_CACHE = None


def kernel(x, wq, bq, wk, bk, wv, bv):
    global _NC_CACHE
    if _NC_CACHE is None:
        _NC_CACHE = build()
    nc = _NC_CACHE
    in_maps = make_in_maps(x, wq, bq, wk, bk, wv, bv)
    res = run_bass_kernel_spmd(nc, in_maps, core_ids=list(range(NCORES)))
    # per-core "out" is [2, 128, NB, S] (m, p, b, s) -> [NB, 256, S]
    out = np.concatenate(
        [res.results[i]["out"].transpose(2, 0, 1, 3).reshape(NB, 256, S)
         for i in range(NCORES)], axis=0)
    return np.ascontiguousarray(out.reshape(B, 256, L, L).astype(np.float32))


# revision 39
# speedup vs baseline: 1.1783x; 1.1783x over previous
"""Trainium2 Bass kernel v3: causal spatial attention block,
minimal-instruction design (f32 everywhere except the fp8-DoubleRow
q/k/scores path).

Data-parallel over batch across 8 NeuronCores (4 batches/core, no
collectives). The execution backend charges a large, mostly
size-independent cost per *instruction* (ACT worst ~250us; MM ~50us;
DVE/DMA ~30-100us), so this kernel minimizes instruction count:
  - Q/K projections (fp8-DR): 32 matmuls total for all 4 batches
    (contraction 256 per instr); fused bias+relu drains write fp8 q/k
    directly -> 4 DVE instructions.
  - V projection per batch (f32, accuracy-critical): 16 matmuls produce
    vT directly in [t, v] layout (lhsT = x-tile, rhs = wv); the free-dim
    bias + relu run as two DVE passes against a broadcast const (cheaper
    than rank-1 bias matmuls on the PE).
  - scores per batch (fp8-DR): per (t-tile, 512-chunk): 1 DR k-matmul +
    1 additive-mask matmul (identity-lhsT x shifted window into one
    [128,1920] E-pattern const); fully-masked chunks are skipped
    entirely (their p region is never contracted by the causally
    trimmed o/dn). exp(score/16) runs as ONE activation per 4-t-tile
    phase over the whole [128,4,1024] PSUM (2 per batch); masked
    entries underflow to exact f32 zeros.
  - o/dn per batch (f32): 36 matmuls, causally trimmed; reciprocal + 2
    multiplies normalize during the PSUM drain into an all-batch output
    tile shipped by ONE DMA per rep (transposed [m,p,b,s] DRAM layout,
    un-transposed on the host).
All PSUM phases share one [128,4096] tile (8 banks), start=True only on
the first matmul touching each 512-col bank. ~470 instructions per
4-batch pass vs 1227 for the previous fp8-everything kernel (77.5ms ->
~21ms measured by reps-delta wall timing; NTFF profiling wedges the
axon terminal).
"""
import numpy as np
from contextlib import ExitStack

import concourse.bass as bass
import concourse.mybir as mybir
import concourse.tile as tile
from concourse import bacc
from concourse.bass_utils import run_bass_kernel_spmd

F32 = mybir.dt.float32
FP8 = mybir.dt.float8e4
AF = mybir.ActivationFunctionType
ALU = mybir.AluOpType
DR = mybir.MatmulPerfMode.DoubleRow

B, C, L, EMB = 32, 224, 32, 16
S = L * L            # 1024
CIN = 256
NCORES = 8
NB = B // NCORES     # 4 batches per core
SB = NB * S          # 4096 columns: all 4 batches side by side
MASKV = -30000.0     # exp((x-30000)/16) underflows to exact f32 zero


def _pos_embeddings() -> np.ndarray:
    pos = np.arange(L)[:, None].astype(np.float64)
    j = np.arange(EMB)[None, :]
    enc = pos / np.power(10000.0, 2.0 * (j // 2) / EMB)
    enc[0, :] = 0.0
    enc[1:, 0::2] = np.sin(enc[1:, 0::2])
    enc[1:, 1::2] = np.cos(enc[1:, 1::2])
    t = enc.astype(np.float32)
    x = np.tile(t.reshape(1, EMB, L, 1), (1, 1, 1, L))
    y = np.tile(t.reshape(1, EMB, 1, L), (1, 1, L, 1))
    pe = np.concatenate((x, y), axis=1)[0]
    return np.ascontiguousarray(pe.reshape(2 * EMB, S))


def build(reps: int = 1):
    nc = bacc.Bacc("TRN2", target_bir_lowering=False, debug=False,
                   num_devices=NCORES)
    x_d = nc.declare_dram_parameter("xm", [2, 128, NB, S], F32, isOutput=False)
    wq_d = nc.declare_dram_parameter("wqt", [CIN, 256], F32, isOutput=False)
    wk_d = nc.declare_dram_parameter("wkt", [CIN, 256], F32, isOutput=False)
    wv_d = nc.declare_dram_parameter("wvt", [CIN, 256], F32, isOutput=False)
    bqk_d = nc.declare_dram_parameter("bqk", [128, 4], F32, isOutput=False)
    bv_d = nc.declare_dram_parameter("bvb", [128, 2048], F32, isOutput=False)
    # mconst: [ identity(128) | E-mask(1920) ]  (E[p][j] = MASKV if j < 896+p)
    mk_d = nc.declare_dram_parameter("mconst", [128, 2048], F32, isOutput=False)
    out_d = nc.declare_dram_parameter("out", [2, 128, NB, S], F32,
                                      isOutput=True)

    with ExitStack() as ctx:
        tc = ctx.enter_context(tile.TileContext(nc))
        const = ctx.enter_context(tc.tile_pool(name="const", bufs=1))
        xp = ctx.enter_context(tc.tile_pool(name="xp", bufs=1))
        qkp = ctx.enter_context(tc.tile_pool(name="qk", bufs=1))
        vtp = ctx.enter_context(tc.tile_pool(name="vt", bufs=2))
        ppool = ctx.enter_context(tc.tile_pool(name="pp", bufs=1))
        op = ctx.enter_context(tc.tile_pool(name="ob", bufs=2))
        rp = ctx.enter_context(tc.tile_pool(name="rp", bufs=2))
        psp = ctx.enter_context(tc.tile_pool(name="ps", bufs=1, space="PSUM"))

        # ---- constants ----
        # SWDGE casting DMAs: f32 DRAM -> fp8 SBUF, bit-exact RNE
        wqt = const.tile([128, 2, 256], FP8, tag="wqt")
        nc.gpsimd.dma_start(wqt[:],
                            wq_d[:].rearrange("(ci p) m -> p ci m", p=128))
        wkt = const.tile([128, 2, 256], FP8, tag="wkt")
        nc.gpsimd.dma_start(wkt[:],
                            wk_d[:].rearrange("(ci p) m -> p ci m", p=128))
        wvt = const.tile([128, 2, 256], F32, tag="wvt")
        nc.sync.dma_start(wvt[:], wv_d[:].rearrange("(ci p) m -> p ci m", p=128))
        bqk = const.tile([128, 4], F32, tag="bqk")
        nc.sync.dma_start(bqk[:], bqk_d[:])
        bvb = const.tile([128, 2048], F32, tag="bvb")
        nc.sync.dma_start(bvb[:], bv_d[:])
        mck = const.tile([128, 2048], F32, tag="mck")
        nc.sync.dma_start(mck[:], mk_d[:])
        ones128 = const.tile([128, 128], F32, tag="ones128")
        nc.vector.memset(ones128[:], 1.0)
        ident = mck[:, 0:128]
        emask = mck[:, 128:2048]           # [128, 1920]

        ps = psp.tile([128, 4096], F32, tag="ps")

        for rep in range(reps):
            # ---- x load: all 4 batches -> [128, 2(ci), 4096] ----
            xt = xp.tile([128, 2, SB], F32, tag="xt")
            nc.sync.dma_start(xt[:], x_d[:].rearrange("ci p b s -> p ci (b s)"))
            # fp8 copy of x for the Q/K path (SWDGE casting DMA)
            xt8 = xp.tile([128, 2, SB], FP8, tag="xt8")
            nc.gpsimd.dma_start(xt8[:],
                                x_d[:].rearrange("ci p b s -> p ci (b s)"))

            # ---- Q/K projections (fp8 DoubleRow), all batches at once ----
            q = qkp.tile([128, 2, SB], FP8, tag="q")
            k = qkp.tile([128, 2, SB], FP8, tag="k")
            for pi, (wt, dst) in enumerate(((wqt, q), (wkt, k))):
                for m in range(2):
                    for c in range(8):
                        cs = slice(512 * c, 512 * c + 512)
                        nc.tensor.matmul(ps[:, cs],
                                         wt[:, :, 128 * m:128 * m + 128],
                                         xt8[:, :, cs], start=True, stop=True,
                                         perf_mode=DR)
                    nc.vector.tensor_scalar(dst[:, m, :], ps[:],
                                            bqk[:, 2 * pi + m:2 * pi + m + 1],
                                            0.0, op0=ALU.add, op1=ALU.max)

            for b in range(NB):
                s0 = b * S

                # ---- V projection -> vT [128(t), 8, 256(v)] ----
                vt = vtp.tile([128, 8, 256], F32, tag="vt")
                for a in range(8):
                    for ci in range(2):
                        nc.tensor.matmul(
                            ps[:, 256 * a:256 * a + 256],
                            xt[:, ci, s0 + 128 * a:s0 + 128 * a + 128],
                            wvt[:, ci, :],
                            start=(a % 2 == 0 and ci == 0),
                            stop=(a % 2 == 1 and ci == 1))
                # bias (along free dim) + relu on DVE: 2 passes replace
                # 4 rank-1 bias matmuls
                nc.vector.tensor_tensor(
                    vt[:], ps[:, 0:2048].rearrange("p (a v) -> p a v", a=8),
                    bvb[:].rearrange("p (a v) -> p a v", a=8), op=ALU.add)
                nc.vector.tensor_scalar(vt[:], vt[:], 0.0, None, op0=ALU.max)

                # ---- scores + exp: 2 phases of 4 t-tiles ----
                # Fully-masked chunks (i>=4, s-chunk 0) are skipped outright:
                # the exp there reads this batch's phase-0 leftovers (score-
                # valued, so exp stays finite) and o/dn never contracts that
                # region.
                pt = ppool.tile([128, 8, S], F32, tag="pt")
                for ph in range(2):
                    for j in range(4):
                        i = 4 * ph + j
                        tc_ = slice(s0 + 128 * i, 128 * i + s0 + 128)
                        for c in range(2):
                            sc = ps[:, 1024 * j + 512 * c:
                                    1024 * j + 512 * c + 512]
                            j0 = 896 - 128 * i + 512 * c
                            if j0 <= 384:
                                continue
                            has_mask = j0 < 1024
                            qs = slice(s0 + 512 * c, s0 + 512 * c + 512)
                            nc.tensor.matmul(sc, k[:, :, tc_],
                                             q[:, :, qs], start=True,
                                             stop=not has_mask,
                                             perf_mode=DR)
                            if has_mask:
                                nc.tensor.matmul(sc, ident,
                                                 emask[:, j0:j0 + 512],
                                                 start=False, stop=True)
                    nc.scalar.activation(
                        pt[:, 4 * ph:4 * ph + 4, :],
                        ps[:].rearrange("p (a s) -> p a s", a=4),
                        AF.Exp, scale=0.0625)

                # ---- o / dn ----
                om0 = ps[:, 0:1024]
                om1 = ps[:, 1024:2048]
                dnp = ps[:, 2048:3072]
                for c in range(2):
                    ntile = 4 * c + 4   # causal: chunk c only sees t < 512(c+1)
                    for i in range(ntile):
                        st, sp_ = (i == 0), (i == ntile - 1)
                        pcs = pt[:, i, 512 * c:512 * c + 512]
                        for roff, lh in ((0, vt[:, i, 0:128]),
                                         (1024, vt[:, i, 128:256]),
                                         (2048, ones128[:])):
                            nc.tensor.matmul(
                                ps[:, roff + 512 * c:roff + 512 * c + 512],
                                lh, pcs, start=st, stop=sp_)

                # ---- normalize ----
                rec = rp.tile([128, S], F32, tag="rec")
                nc.vector.reciprocal(rec[:], dnp)
                if b == 0:
                    osb = op.tile([128, 2, SB], F32, tag="osb")
                nc.vector.tensor_tensor(osb[:, 0, s0:s0 + S], om0, rec[:],
                                        op=ALU.mult)
                nc.vector.tensor_tensor(osb[:, 1, s0:s0 + S], om1, rec[:],
                                        op=ALU.mult)
            # one output DMA for all 4 batches
            nc.sync.dma_start(out_d[:].rearrange("m p b s -> p m (b s)"),
                              osb[:])

    nc.finalize()
    return nc


def make_in_maps(x, wq, bq, wk, bk, wv, bv):
    x_r = x.reshape(B, C, S).astype(np.float32)
    pe = _pos_embeddings()
    xm = np.concatenate(
        [x_r, np.broadcast_to(pe[None], (B, 2 * EMB, S))], axis=1)
    # [B, 2, 128, S] -> per-core [2, 128, NB, S] so the on-device DMA can
    # group (b s) into adjacent columns
    xm = xm.reshape(B, 2, 128, S)
    wqt = np.ascontiguousarray(wq.T.astype(np.float32))
    wkt = np.ascontiguousarray(wk.T.astype(np.float32))
    wvt = np.ascontiguousarray(wv.T.astype(np.float32))
    bq = bq.astype(np.float32)
    bk = bk.astype(np.float32)
    bqk = np.ascontiguousarray(
        np.stack([bq[:128], bq[128:], bk[:128], bk[128:]], axis=1))
    bvb = np.ascontiguousarray(np.broadcast_to(
        np.tile(bv.astype(np.float32), 8)[None, :], (128, 2048)))
    ident = np.eye(128, dtype=np.float32)
    jj = np.arange(1920)[None, :]
    ppn = np.arange(128)[:, None]
    emask = np.where(jj < 896 + ppn, np.float32(MASKV), np.float32(0.0))
    mconst = np.ascontiguousarray(
        np.concatenate([ident, emask.astype(np.float32)], axis=1))
    common = dict(wqt=wqt, wkt=wkt, wvt=wvt, bqk=bqk, bvb=bvb, mconst=mconst)
    return [dict(xm=np.ascontiguousarray(
                     xm[i * NB:(i + 1) * NB].transpose(1, 2, 0, 3)), **common)
            for i in range(NCORES)]


_NC_CACHE = None


def kernel(x, wq, bq, wk, bk, wv, bv):
    global _NC_CACHE
    if _NC_CACHE is None:
        _NC_CACHE = build()
    nc = _NC_CACHE
    in_maps = make_in_maps(x, wq, bq, wk, bk, wv, bv)
    res = run_bass_kernel_spmd(nc, in_maps, core_ids=list(range(NCORES)))
    # per-core "out" is [2, 128, NB, S] (m, p, b, s) -> [NB, 256, S]
    out = np.concatenate(
        [res.results[i]["out"].transpose(2, 0, 1, 3).reshape(NB, 256, S)
         for i in range(NCORES)], axis=0)
    return np.ascontiguousarray(out.reshape(B, 256, L, L).astype(np.float32))
